# revision 4
# baseline (speedup 1.0000x reference)
"""Trainium2 Bass kernel for nn_AttentionLayer (scatter_memory).

Computes, for each of U=256 units (sharded 32/core across 8 cores):
    query  = attention @ W[u]                  [B, D_OUT]
    logits = query @ keys[u].T / temp[u]       [B, C]
    weights= softmax(where(mask, logits,-inf)) [B, C]
    w      = weights * rewards_mem[u]; rewards = w.sum(-1); w /= rewards
    outputs= w @ values[u]                     [B, D_V]

Key tricks:
  - exp without max-subtraction (logits ~ N(0,1)); mask applied as f32 multiply,
    so the softmax denominator algebra is: EM = exp*mask, s1 = sum(EM),
    EMR = EM*r, s2 = sum(EMR), weights = EM/s1, rewards = s2/s1,
    outputs = (EMR @ values)/s2.
  - keys/W/values are shipped as bf16 arrays of shape [.., 2*64] with the
    real (rounded) values in odd slots and zeros in even slots.  This makes
    the DMA-xbar transpose (bf16-only, free dim must be %128) legal for keys,
    and makes W/values load as interleaved matmul operands whose even
    rows/cols contribute exact zeros.
  - 4 units per round are packed into the PE array via tile_position
    (M=32 each, col groups 0..3).
"""

import os
import sys

sys.path.insert(0, "/opt/trn_rl_repo")

from contextlib import ExitStack

import numpy as np
import ml_dtypes

import concourse.bass as bass
import concourse.tile as tile
from concourse import bacc, mybir
from concourse.bass_utils import run_bass_kernel_spmd

BF = ml_dtypes.bfloat16

B, U, D_IN, D_OUT, C, D_V = 32, 256, 512, 64, 1024, 64
N_CORES = 8
UPC = U // N_CORES          # 32 units per core
GRP = 4                     # units per round (PE col groups)
ROUNDS = UPC // GRP         # 8

FP32 = mybir.dt.float32
BF16 = mybir.dt.bfloat16
U8 = mybir.dt.uint8

LAST_RESULTS = None         # BassKernelResults of the most recent run


def _build_body(ctx: ExitStack, tc, io):
    nc = tc.nc
    (keysz, wz, valsz, maskp, rewp, attnT, invtp, indp, identp,
     out_o, out_w, out_r) = io

    # ---- pools ----
    const = ctx.enter_context(tc.tile_pool(name="const", bufs=1))
    dpool = ctx.enter_context(tc.tile_pool(name="dma", bufs=2))
    spool = ctx.enter_context(tc.tile_pool(name="sm", bufs=2))
    tiny = ctx.enter_context(tc.tile_pool(name="tiny", bufs=2))
    pp = ctx.enter_context(tc.tile_pool(name="ppool", bufs=1, space="PSUM"))
    pp2 = ctx.enter_context(tc.tile_pool(name="ppool2", bufs=2, space="PSUM"))

    # ---- constants (loaded once) ----
    attn_sb = const.tile([128, GRP * B], BF16)          # [dp, (dc b)]
    nc.sync.dma_start(attn_sb[:], attnT.rearrange("(a p) b -> p a b", p=128))
    invt_sb = const.tile([128, ROUNDS], FP32)
    nc.sync.dma_start(invt_sb[:], invtp[:, :])
    ind_sb = const.tile([GRP, 128], BF16)
    nc.sync.dma_start(ind_sb[:], indp[:, :])
    ident_sb = const.tile([128, 128], FP32)
    nc.sync.dma_start(ident_sb[:], identp[:, :])

    outbuf = const.tile([128, ROUNDS * D_V], FP32)      # [(j b), (r v)]
    rewbuf = const.tile([128, ROUNDS], FP32)            # [(j b), r]

    # DRAM views with (u b) stacked partitions
    mask_r = maskp.rearrange("b u c -> u b c")
    wout_r = out_w.rearrange("b u c -> u b c")

    for r in range(ROUNDS):
        # ---------------- DMAs ----------------
        kts, wvs, vls = [], [], []
        for j in range(GRP):
            u = GRP * r + j
            kt = dpool.tile([128, C], BF16, tag=f"kt{j}")
            nc.sync.dma_start(kt[:], keysz[u], transpose=True)
            kts.append(kt)
            wv = dpool.tile([128, GRP * 128], BF16, tag=f"wv{j}")
            nc.sync.dma_start(wv[:], wz[u].rearrange("(a p) o -> p a o", p=128))
            wvs.append(wv)
            vl = dpool.tile([128, 8 * 128], BF16, tag=f"vl{j}")
            nc.sync.dma_start(vl[:], valsz[u].rearrange("(a p) v -> p a v", p=128))
            vls.append(vl)
        mk = dpool.tile([128, C], FP32, tag="mk")
        nc.gpsimd.dma_start(mk[:], mask_r[GRP * r:GRP * (r + 1)])
        r4 = dpool.tile([GRP, C], BF16, tag="r4")
        nc.gpsimd.dma_start(r4[:], rewp[GRP * r:GRP * (r + 1), :])

        # ---------------- query: qT (interleaved, even rows = 0) ----------------
        qt_ps = pp.tile([128, GRP * B], FP32, tag="qt")
        for j in range(GRP):
            for dc in range(4):
                nc.tensor.matmul(
                    qt_ps[:, B * j:B * (j + 1)],
                    wvs[j][:, 128 * dc:128 * (dc + 1)],
                    attn_sb[:, B * dc:B * (dc + 1)],
                    start=(dc == 0), stop=(dc == 3),
                )
        qb = tiny.tile([128, GRP * B], BF16, tag="qb")
        nc.vector.tensor_copy(qb[:], qt_ps[:])

        # ---------------- rewards broadcast: rb[p, c] = r[unit(p), c] ----------------
        rb_ps = pp.tile([128, C], FP32, tag="rb")
        for h in range(2):
            nc.tensor.matmul(
                rb_ps[:, 512 * h:512 * (h + 1)],
                ind_sb[:, :],
                r4[:, 512 * h:512 * (h + 1)],
                start=True, stop=True,
            )

        # ---------------- logits ----------------
        lg_ps = pp.tile([128, C], FP32, tag="lg")
        for j in range(GRP):
            for h in range(2):
                nc.tensor.matmul(
                    lg_ps[B * j:B * (j + 1), 512 * h:512 * (h + 1)],
                    qb[:, B * j:B * (j + 1)],
                    kts[j][:, 512 * h:512 * (h + 1)],
                    start=True, stop=True,
                    tile_position=(0, B * j),
                )

        # ---------------- exp (scaled by 1/temp), mask ----------------
        ex = spool.tile([128, C], FP32, tag="ex")
        nc.scalar.activation(ex[:], lg_ps[:],
                             mybir.ActivationFunctionType.Exp,
                             scale=invt_sb[:, r:r + 1])
        em = spool.tile([128, C], FP32, tag="em")
        nc.vector.tensor_tensor(em[:], ex[:], mk[:], op=mybir.AluOpType.mult)
        emr = spool.tile([128, C], FP32, tag="emr")
        nc.vector.tensor_tensor(emr[:], em[:], rb_ps[:], op=mybir.AluOpType.mult)

        # ---------------- transpose EMR -> [c, (j b)] bf16 ----------------
        emrt = spool.tile([128, C], BF16, tag="emrt")
        for half in range(4):
            tp_ps = pp2.tile([128, 256], FP32, tag="tp")
            for q in range(2):
                cc = 2 * half + q
                nc.tensor.transpose(
                    tp_ps[:, 128 * q:128 * (q + 1)],
                    emr[:, 128 * cc:128 * (cc + 1)],
                    ident_sb[:],
                )
            nc.scalar.activation(emrt[:, 256 * half:256 * (half + 1)], tp_ps[:],
                                 mybir.ActivationFunctionType.Copy)

        # ---------------- outputs: (EMR @ values) / s2 ----------------
        o_ps = pp.tile([128, 2 * D_V], FP32, tag="o")
        for j in range(GRP):
            for cc in range(8):
                nc.tensor.matmul(
                    o_ps[B * j:B * (j + 1), :],
                    emrt[:, 128 * cc + B * j:128 * cc + B * (j + 1)],
                    vls[j][:, 128 * cc:128 * (cc + 1)],
                    start=(cc == 0), stop=(cc == 7),
                    tile_position=(0, B * j),
                )
        # o_ps columns: 0 -> s2 (ones slot), 2 -> s1 (1/r slot), odd -> EMR@values
        s2i = tiny.tile([128, 1], FP32, tag="s2i")
        nc.vector.reciprocal(s2i[:], o_ps[:, 0:1])
        s1i = tiny.tile([128, 1], FP32, tag="s1i")
        nc.vector.reciprocal(s1i[:], o_ps[:, 2:3])
        wout = spool.tile([128, C], FP32, tag="wout")
        nc.vector.tensor_scalar_mul(wout[:], em[:], s1i[:])
        nc.sync.dma_start(wout_r[GRP * r:GRP * (r + 1)], wout[:])
        nc.vector.tensor_tensor(rewbuf[:, r:r + 1], o_ps[:, 0:1], s1i[:],
                                op=mybir.AluOpType.mult)
        nc.vector.tensor_scalar_mul(
            outbuf[:, D_V * r:D_V * (r + 1)],
            o_ps[:, 1:2 * D_V:2],
            s2i[:],
        )

    # ---------------- final stores ----------------
    nc.sync.dma_start(
        out_o.rearrange("b (r j) v -> j b r v", j=GRP), outbuf[:])
    nc.sync.dma_start(
        out_r.rearrange("b (r j) -> j b r", j=GRP), rewbuf[:])


_NC_CACHE = None


def _get_nc():
    global _NC_CACHE
    if _NC_CACHE is not None:
        return _NC_CACHE
    nc = bacc.Bacc("TRN2", target_bir_lowering=False, debug=False,
                   num_devices=N_CORES)
    keysz = nc.declare_dram_parameter("keysz", [UPC, C, 2 * D_OUT], BF16, isOutput=False).ap()
    wz = nc.declare_dram_parameter("wz", [UPC, D_IN, 2 * D_OUT], BF16, isOutput=False).ap()
    valsz = nc.declare_dram_parameter("valsz", [UPC, C, 2 * D_V], BF16, isOutput=False).ap()
    maskp = nc.declare_dram_parameter("maskp", [B, UPC, C], U8, isOutput=False).ap()
    rewp = nc.declare_dram_parameter("rewp", [UPC, C], FP32, isOutput=False).ap()
    attnT = nc.declare_dram_parameter("attnT", [D_IN, B], BF16, isOutput=False).ap()
    invtp = nc.declare_dram_parameter("invtp", [128, ROUNDS], FP32, isOutput=False).ap()
    indp = nc.declare_dram_parameter("indp", [GRP, 128], BF16, isOutput=False).ap()
    identp = nc.declare_dram_parameter("identp", [128, 128], FP32, isOutput=False).ap()
    out_o = nc.declare_dram_parameter("out_outputs", [B, UPC, D_V], FP32, isOutput=True).ap()
    out_w = nc.declare_dram_parameter("out_weights", [B, UPC, C], FP32, isOutput=True).ap()
    out_r = nc.declare_dram_parameter("out_rewards", [B, UPC], FP32, isOutput=True).ap()
    io = (keysz, wz, valsz, maskp, rewp, attnT, invtp, indp, identp,
          out_o, out_w, out_r)
    with tile.TileContext(nc) as tc:
        with ExitStack() as ctx:
            _build_body(ctx, tc, io)
    nc.compile()
    _NC_CACHE = nc
    return nc


def _prep_in_maps(attention, W, temperature, keys, values, rewards_mem, mask):
    attention = np.asarray(attention, np.float32)
    W = np.asarray(W, np.float32)
    temperature = np.asarray(temperature, np.float32)
    keys = np.asarray(keys, np.float32)
    values = np.asarray(values, np.float32)
    rewards_mem = np.asarray(rewards_mem, np.float32)
    mask = np.asarray(mask)

    attnT = np.ascontiguousarray(attention.T).astype(BF)
    invt = (1.0 / temperature).astype(np.float32)
    ident = np.eye(128, dtype=np.float32)
    ind = np.zeros((GRP, 128), BF)
    for j in range(GRP):
        ind[j, 32 * j:32 * (j + 1)] = 1

    in_maps = []
    for core in range(N_CORES):
        us = slice(core * UPC, (core + 1) * UPC)
        kz = np.zeros((UPC, C, 2 * D_OUT), BF)
        kz[:, :, 1::2] = keys[us].astype(BF)
        wzv = np.zeros((UPC, D_IN, 2 * D_OUT), BF)
        wzv[:, :, 1::2] = W[us].astype(BF)
        vz = np.zeros((UPC, C, 2 * D_V), BF)
        vz[:, :, 1::2] = values[us].astype(BF)
        vz[:, :, 0] = 1.0                      # -> col 0 of o_ps = sum(EMR) = s2
        vz[:, :, 2] = (1.0 / rewards_mem[us]).astype(BF)  # -> col 2 = sum(EM) = s1
        mk = np.ascontiguousarray(mask[:, us, :]).view(np.uint8)
        # invt_t[32*j + b, r] = 1/temp[core*32 + 4*r + j]
        loc = invt[us].reshape(ROUNDS, GRP)          # [r, j]
        invt_t = np.ascontiguousarray(np.repeat(loc.T, 32, axis=0))  # [128, r]
        in_maps.append(dict(
            keysz=kz, wz=wzv, valsz=vz, maskp=mk,
            rewp=np.ascontiguousarray(rewards_mem[us]),
            attnT=attnT, invtp=invt_t, indp=ind, identp=ident,
        ))
    return in_maps


def kernel(attention, W, temperature, keys, values, rewards_mem, mask,
           trace=False):
    global LAST_RESULTS
    nc = _get_nc()
    in_maps = _prep_in_maps(attention, W, temperature, keys, values,
                            rewards_mem, mask)
    res = run_bass_kernel_spmd(nc, in_maps, core_ids=list(range(N_CORES)),
                               trace=trace)
    LAST_RESULTS = res
    outs = res.results
    outputs = np.concatenate([np.asarray(o["out_outputs"]) for o in outs], axis=1)
    weights = np.concatenate([np.asarray(o["out_weights"]) for o in outs], axis=1)
    rewards = np.concatenate([np.asarray(o["out_rewards"]) for o in outs], axis=1)
    return outputs.astype(np.float32), weights.astype(np.float32), rewards.astype(np.float32)
